# revision 4
# baseline (speedup 1.0000x reference)
"""Trainium2 Bass kernel for nn_Discriminator (GRU-like recurrent discriminator).

Math (per batch row):
    belta = exp(-relu(td @ Wb^T + bb))                       # (T, H)
    for t in 0..T-1:
        s = belta[t] * s
        u = sigmoid(s @ W1h^T + x[t] @ W1x^T + b1)
        r = sigmoid(s @ W2h^T + x[t] @ W2x^T + b2)
        n = tanh((r*s) @ W3h^T + x[t] @ W3x^T + b3)
        s = (1-u)*s + u*n
    out = sigmoid(s @ Wo^T + bo)

Strategy: data-parallel over 8 cores on the batch dim (B=256 -> 32/core).
The x-contributions of all gates and belta are precomputed blockwise
(TS=8 steps per block) into SBUF ring buffers; the Tile list-scheduler
interleaves these "phase 1" matmuls into the recurrence's dependency
stalls.  All gate matmuls run in fp8 (e4m3) DoubleRow mode: contraction
256 per instruction at 0.5 cycles/row, with power-of-2 scaling folded
into the activation `scale` for exact recovery.  The f32 state-carry
path is preserved (only matmul operands are quantized).
"""

import numpy as np
import ml_dtypes

B, T, IN, H = 256, 96, 512, 1024
NCORES = 8
BS = B // NCORES      # 32 batch rows per core
HC = H // 128         # 8 hidden chunks
KC = IN // 128        # 4 input chunks
CB = HC * BS          # 256 packed columns: col = chunk*BS + b
H2 = CB // 2          # 128 packed cols per half
KH = HC // 2          # 4 chunks per half

TS = 8                # time steps per phase-1 block
SC = TS * BS          # 256 psum cols per phase-1 tile
NSLOT = 3             # ring slots

# fp8 scaling (ml_dtypes.float8_e4m3: max 240)
SW = 2.0 ** 12        # recurrent + x weights scale
SS = 2.0 ** 7         # state scale (|s|<=1 -> <=128)
SX = 2.0 ** 5         # x scale (randn, clip)
STD = 2.0 ** 7        # time_delta scale ([0,1))
RING_SCALE = SW * SS  # 2^19: ring pre-activations stored at this scale
PS1_URN = SW * SX     # 2^17: phase-1 psum scale for u/r/n jobs
PS1_B = SW * STD      # 2^19: phase-1 psum scale for belta job

BF16 = ml_dtypes.bfloat16
FP8 = ml_dtypes.float8_e4m3


def build_program(t_steps=T):
    import concourse.mybir as mybir
    import concourse.tile as tile
    from concourse import bacc
    from concourse.masks import make_identity

    f32 = mybir.dt.float32
    bf16 = mybir.dt.bfloat16
    f8 = mybir.dt.float8e4
    AF = mybir.ActivationFunctionType
    ALU = mybir.AluOpType
    DR = mybir.MatmulPerfMode.DoubleRow
    TB = t_steps * BS
    NS = t_steps // TS    # number of phase-1 blocks

    nc = bacc.Bacc("TRN2", target_bir_lowering=False)

    # ---- DRAM I/O (per core; weights replicated by the host) ----
    xt = nc.dram_tensor("xt", [KC, 128, TB], f8, kind="ExternalInput")
    tdt = nc.dram_tensor("tdt", [KC, 128, TB], f8, kind="ExternalInput")
    # recurrent weights, fp8 DoubleRow layout [p, d, c2, m*128+j]
    w1h = nc.dram_tensor("w1h", [128, 2, KH, H], f8, kind="ExternalInput")
    w2h = nc.dram_tensor("w2h", [128, 2, KH, H], f8, kind="ExternalInput")
    w3h = nc.dram_tensor("w3h", [128, 2, KH, H], f8, kind="ExternalInput")
    # x weights, fp8 DoubleRow layout (KC/2 = 2 double-chunks)
    w1x = nc.dram_tensor("w1x", [128, 2, KC // 2, H], f8, kind="ExternalInput")
    w2x = nc.dram_tensor("w2x", [128, 2, KC // 2, H], f8, kind="ExternalInput")
    w3x = nc.dram_tensor("w3x", [128, 2, KC // 2, H], f8, kind="ExternalInput")
    wbt = nc.dram_tensor("wbt", [128, 2, KC // 2, H], f8, kind="ExternalInput")
    # biases: b1..b3 pre-scaled by RING_SCALE; bbn = -bb (true scale)
    b1t = nc.dram_tensor("b1t", [128, HC], f32, kind="ExternalInput")
    b2t = nc.dram_tensor("b2t", [128, HC], f32, kind="ExternalInput")
    b3t = nc.dram_tensor("b3t", [128, HC], f32, kind="ExternalInput")
    bbn = nc.dram_tensor("bbn", [128, HC], f32, kind="ExternalInput")
    wot = nc.dram_tensor("wot", [128, HC], f32, kind="ExternalInput")
    bot = nc.dram_tensor("bot", [1, 1], f32, kind="ExternalInput")
    out = nc.dram_tensor("out", [BS, 1], f32, kind="ExternalOutput")

    with tile.TileContext(nc) as tc:
        with (
            tc.tile_pool(name="singles", bufs=1) as S,
            tc.tile_pool(name="scp", bufs=2) as scp,
            tc.tile_pool(name="ps2", bufs=1, space="PSUM") as ps2,
            tc.tile_pool(name="ps1p", bufs=3, space="PSUM") as ps1p,
        ):
            # ---- persistent SBUF ----
            sw1h = S.tile([128, 2, KH, H], f8)
            sw2h = S.tile([128, 2, KH, H], f8)
            sw3h = S.tile([128, 2, KH, H], f8)
            sw1x = S.tile([128, 2, KC // 2, H], f8)
            sw2x = S.tile([128, 2, KC // 2, H], f8)
            sw3x = S.tile([128, 2, KC // 2, H], f8)
            swbt = S.tile([128, 2, KC // 2, H], f8)
            sb1 = S.tile([128, HC], f32)
            sb2 = S.tile([128, HC], f32)
            sb3 = S.tile([128, HC], f32)
            sbbn = S.tile([128, HC], f32)
            swo = S.tile([128, HC], f32)
            sbo = S.tile([1, 1], f32)
            ident = S.tile([128, 128], bf16)
            make_identity(nc, ident)

            # rings: gate x-contributions (scaled by RING_SCALE) + belta
            ring_u = S.tile([128, NSLOT, TS, CB], bf16)
            ring_r = S.tile([128, NSLOT, TS, CB], bf16)
            ring_n = S.tile([128, NSLOT, TS, CB], bf16)
            ring_b = S.tile([128, NSLOT, TS, CB], f32)
            # x/td stream rings (fp8, scaled)
            xr = S.tile([128, NSLOT, KC, SC], f8)
            tdr = S.tile([128, NSLOT, KC, SC], f8)

            # state: f32 carry + fp8 decayed state (scaled by SS)
            st_lo = S.tile([128, H2], f32)
            st_hi = S.tile([128, H2], f32)
            nc.vector.memset(st_lo, 0.0)
            nc.vector.memset(st_hi, 0.0)

            # ---- upfront DMAs ----
            nc.sync.dma_start(out=sbbn, in_=bbn[:, :])
            nc.sync.dma_start(out=swbt, in_=wbt[:, :, :, :])
            nc.sync.dma_start(out=sb1, in_=b1t[:, :])
            nc.sync.dma_start(out=sb2, in_=b2t[:, :])
            nc.sync.dma_start(out=sb3, in_=b3t[:, :])
            nc.sync.dma_start(out=sw1x, in_=w1x[:, :, :, :])
            nc.sync.dma_start(out=sw2x, in_=w2x[:, :, :, :])
            nc.sync.dma_start(out=sw3x, in_=w3x[:, :, :, :])
            nc.sync.dma_start(out=sw1h, in_=w1h[:, :, :, :])
            nc.sync.dma_start(out=sw2h, in_=w2h[:, :, :, :])
            nc.sync.dma_start(out=sw3h, in_=w3h[:, :, :, :])
            nc.sync.dma_start(out=swo, in_=wot[:, :])
            nc.sync.dma_start(out=sbo, in_=bot[:, :])

            # ---- phase-1 machinery ----
            def block_dmas(s):
                sl = s % NSLOT
                for k in range(KC):
                    nc.sync.dma_start(
                        out=tdr[:, sl, k, :], in_=tdt[k, :, s * SC:(s + 1) * SC]
                    )
                    nc.sync.dma_start(
                        out=xr[:, sl, k, :], in_=xt[k, :, s * SC:(s + 1) * SC]
                    )

            # jobs: (weight, rhs ring, ring out, bias/act info)
            def emit_unit(s, jobi, m):
                """One m-chunk of one job of block s: 2 DR matmuls + post."""
                sl = s % NSLOT
                ps = ps1p.tile([128, SC], f32, tag="ps1", name="ps1")
                wsb, rin = (
                    (swbt, tdr), (sw1x, xr), (sw2x, xr), (sw3x, xr)
                )[jobi]
                for c2 in range(KC // 2):
                    nc.tensor.matmul(
                        ps,
                        wsb[:, :, c2, m * 128:(m + 1) * 128],
                        rin[:, sl, 2 * c2:2 * c2 + 2, :],
                        start=(c2 == 0), stop=(c2 == KC // 2 - 1),
                        perf_mode=DR,
                    )
                ps3 = ps.rearrange("p (t b) -> p t b", b=BS)
                if jobi == 0:
                    # belta = min(1, exp(-(z + bb))); z = ps / PS1_B
                    bview = ring_b[:, sl, :, m * BS:(m + 1) * BS]
                    nc.scalar.activation(
                        bview, ps3, AF.Exp,
                        bias=sbbn[:, m:m + 1], scale=-1.0 / PS1_B,
                    )
                    nc.gpsimd.tensor_scalar_min(bview, bview, 1.0)
                else:
                    bias = (None, sb1, sb2, sb3)[jobi]
                    oview = (None, ring_u, ring_r, ring_n)[jobi][
                        :, sl, :, m * BS:(m + 1) * BS
                    ]
                    # ring = ps * (RING_SCALE/PS1_URN) + bias(scaled);
                    # gpsimd can't read PSUM -> u on ACT, r/n on DVE
                    if jobi == 1:
                        nc.scalar.activation(
                            oview, ps3, AF.Identity,
                            bias=bias[:, m:m + 1], scale=RING_SCALE / PS1_URN,
                        )
                    else:
                        nc.vector.tensor_scalar(
                            oview, ps3, RING_SCALE / PS1_URN, bias[:, m:m + 1],
                            op0=ALU.mult, op1=ALU.add,
                        )

            def feed_block_units(s, lo, hi):
                """Emit units [lo, hi) of block s. Unit = jobi*HC + m,
                belta job first."""
                for ui in range(lo, hi):
                    emit_unit(s, ui // HC, ui % HC)

            # ---- prologue: blocks 0 and 1 ----
            block_dmas(0)
            block_dmas(1)
            feed_block_units(0, 0, 4 * HC)
            feed_block_units(1, 0, 4 * HC)

            # ---- recurrence ----
            # fp8 decayed state (scaled SS) and fp8 r*state
            sb8 = S.tile([128, HC, BS], f8)
            nc.vector.memset(sb8, 0.0)
            sb8f = sb8.rearrange("p c b -> p (c b)")

            for t in range(t_steps):
                sl = (t // TS) % NSLOT
                tt = t % TS
                ru = ring_u[:, sl, tt, :]
                rr = ring_r[:, sl, tt, :]
                rn = ring_n[:, sl, tt, :]

                psr = ps2.tile([128, CB], f32, tag="psr", name="psr")
                psu_lo = ps2.tile([128, H2], f32, tag="psu_lo", name="psu_lo")
                psu_hi = ps2.tile([128, H2], f32, tag="psu_hi", name="psu_hi")
                psn_lo = ps2.tile([128, H2], f32, tag="psn_lo", name="psn_lo")
                psn_hi = ps2.tile([128, H2], f32, tag="psn_hi", name="psn_hi")

                # inject x-contributions (+bias), all scaled RING_SCALE
                nc.tensor.matmul(psr, ident, rr, start=True, stop=False)
                nc.tensor.matmul(psu_lo, ident, ru[:, :H2], start=True, stop=False)
                nc.tensor.matmul(psu_hi, ident, ru[:, H2:], start=True, stop=False)
                nc.tensor.matmul(psn_lo, ident, rn[:, :H2], start=True, stop=False)
                nc.tensor.matmul(psn_hi, ident, rn[:, H2:], start=True, stop=False)

                # r gate: c2-outer so the low state half unblocks it
                for c2 in range(KH):
                    for m in range(HC):
                        nc.tensor.matmul(
                            psr[:, m * BS:(m + 1) * BS],
                            sw2h[:, :, c2, m * 128:(m + 1) * 128],
                            sb8[:, 2 * c2:2 * c2 + 2, :],
                            start=False,
                            stop=(c2 == KH - 1 and m == HC - 1),
                            perf_mode=DR,
                        )
                rg = scp.tile([128, CB], bf16, tag="rg", name="rg")
                nc.scalar.activation(
                    rg, psr, AF.Sigmoid, scale=1.0 / RING_SCALE
                )
                rs8 = scp.tile([128, HC, BS], f8, tag="rs8", name="rs8")
                rs8f = rs8.rearrange("p c b -> p (c b)")
                nc.vector.tensor_mul(rs8f, rg, sb8f)

                # u gate: low-half psum group completes first
                for mg in range(2):
                    for c2 in range(KH):
                        for m in range(mg * KH, mg * KH + KH):
                            nc.tensor.matmul(
                                (psu_lo, psu_hi)[mg][
                                    :, (m - mg * KH) * BS:(m - mg * KH + 1) * BS
                                ],
                                sw1h[:, :, c2, m * 128:(m + 1) * 128],
                                sb8[:, 2 * c2:2 * c2 + 2, :],
                                start=False,
                                stop=(c2 == KH - 1 and m == mg * KH + KH - 1),
                                perf_mode=DR,
                            )
                # n gate (rhs = r*s)
                for mg in range(2):
                    for c2 in range(KH):
                        for m in range(mg * KH, mg * KH + KH):
                            nc.tensor.matmul(
                                (psn_lo, psn_hi)[mg][
                                    :, (m - mg * KH) * BS:(m - mg * KH + 1) * BS
                                ],
                                sw3h[:, :, c2, m * 128:(m + 1) * 128],
                                rs8[:, 2 * c2:2 * c2 + 2, :],
                                start=False,
                                stop=(c2 == KH - 1 and m == mg * KH + KH - 1),
                                perf_mode=DR,
                            )

                last = t == t_steps - 1
                if not last:
                    t1 = t + 1
                    rb = ring_b[:, (t1 // TS) % NSLOT, t1 % TS, :]

                for half in range(2):
                    lo, hi = half * H2, (half + 1) * H2
                    psu = (psu_lo, psu_hi)[half]
                    psn = (psn_lo, psn_hi)[half]
                    st = (st_lo, st_hi)[half]
                    ug = scp.tile([128, H2], bf16, tag=f"ug{half}", name="ug")
                    nc.scalar.activation(
                        ug, psu, AF.Sigmoid, scale=1.0 / RING_SCALE
                    )
                    # stm = belta*state (f32, true scale) on gpsimd
                    if not last:
                        stm = scp.tile([128, H2], f32, tag=f"stm{half}", name="stm")
                        nc.gpsimd.tensor_mul(stm, st, rb[:, lo:hi])
                    ng = scp.tile([128, H2], bf16, tag=f"ng{half}", name="ng")
                    nc.scalar.activation(
                        ng, psn, AF.Tanh, scale=1.0 / RING_SCALE
                    )
                    e = scp.tile([128, H2], bf16, tag=f"e{half}", name="e")
                    nc.vector.tensor_mul(e, ug, ng)
                    if last:
                        # st = e - (ug-1)*st  [in-place final state]
                        wneg = scp.tile([128, H2], f32, tag=f"wn{half}", name="wneg")
                        nc.vector.scalar_tensor_tensor(
                            wneg, ug, 1.0, st, op0=ALU.subtract, op1=ALU.mult
                        )
                        nc.vector.tensor_sub(st, e, wneg)
                    else:
                        # wneg = (ug-1)*stm ; st' = e - wneg ; sb8' = (st'*SS)*rb
                        wneg = scp.tile([128, H2], f32, tag=f"wn{half}", name="wneg")
                        nc.vector.scalar_tensor_tensor(
                            wneg, ug, 1.0, stm, op0=ALU.subtract, op1=ALU.mult
                        )
                        nc.vector.tensor_sub(st, e, wneg)
                        nc.vector.scalar_tensor_tensor(
                            sb8f[:, lo:hi], st, SS, rb[:, lo:hi],
                            op0=ALU.mult, op1=ALU.mult,
                        )

                # ---- feed phase-1 blocks into this step's stalls ----
                fb = t // TS + 2
                if fb < NS:
                    ph = t % TS
                    if ph == 0:
                        block_dmas(fb)
                    feed_block_units(fb, ph * 4, ph * 4 + 4)

            # ---- head: out = sigmoid(s @ Wo^T + bo) ----
            pso = ps2.tile([1, BS], f32, tag="psr", name="pso")
            stl3 = st_lo.rearrange("p (c b) -> p c b", b=BS)
            sth3 = st_hi.rearrange("p (c b) -> p c b", b=BS)
            for k in range(HC):
                src = stl3[:, k, :] if k < KH else sth3[:, k - KH, :]
                nc.tensor.matmul(
                    pso, swo[:, k:k + 1], src,
                    start=(k == 0), stop=(k == HC - 1),
                )
            ob = scp.tile([1, BS], f32, tag="ob", name="ob")
            nc.scalar.activation(ob, pso, AF.Sigmoid, bias=sbo[0:1, 0:1])
            nc.sync.dma_start(out=out[:, :], in_=ob)

    nc.finalize()
    return nc


def _f8(a, clip=224.0):
    return np.clip(a, -clip, clip).astype(FP8)


def _pack_wh8(w):
    # [H, H] -> [128, 2, KH, H]; out[p,d,c2,m*128+j] = w[m*128+j,(2c2+d)*128+p]*SW
    t = (w * SW).reshape(HC, 128, KH, 2, 128).transpose(4, 3, 2, 0, 1)
    return _f8(np.ascontiguousarray(t.reshape(128, 2, KH, H)))


def _pack_wx8(w):
    # [H, IN] -> [128, 2, KC//2, H]
    t = (w * SW).reshape(HC, 128, KC // 2, 2, 128).transpose(4, 3, 2, 0, 1)
    return _f8(np.ascontiguousarray(t.reshape(128, 2, KC // 2, H)))


def _pack_bias(b, scale=1.0):  # [H] -> [128, HC]
    return np.ascontiguousarray((b * scale).reshape(HC, 128).T).astype(np.float32)


def _pack_x8(xs, t_steps, scale):  # [BS, t, IN] -> [KC, 128, t*BS]
    t = (xs * scale).reshape(BS, t_steps, KC, 128).transpose(2, 3, 1, 0)
    return _f8(np.ascontiguousarray(t.reshape(KC, 128, -1)))


def prepare_in_maps(x, time_delta, Wb, bb, W1, b1, W2, b2, W3, b3, Wo, bo,
                    t_steps=T, ncores=NCORES):
    x = np.asarray(x, np.float32)
    time_delta = np.asarray(time_delta, np.float32)
    common = {
        "w1h": _pack_wh8(np.asarray(W1, np.float32)[:, :H]),
        "w2h": _pack_wh8(np.asarray(W2, np.float32)[:, :H]),
        "w3h": _pack_wh8(np.asarray(W3, np.float32)[:, :H]),
        "w1x": _pack_wx8(np.asarray(W1, np.float32)[:, H:]),
        "w2x": _pack_wx8(np.asarray(W2, np.float32)[:, H:]),
        "w3x": _pack_wx8(np.asarray(W3, np.float32)[:, H:]),
        "wbt": _pack_wx8(np.asarray(Wb, np.float32)),
        "b1t": _pack_bias(np.asarray(b1, np.float32), RING_SCALE),
        "b2t": _pack_bias(np.asarray(b2, np.float32), RING_SCALE),
        "b3t": _pack_bias(np.asarray(b3, np.float32), RING_SCALE),
        "bbn": _pack_bias(-np.asarray(bb, np.float32)),
        "wot": _pack_bias(np.asarray(Wo, np.float32).reshape(H)),
        "bot": np.asarray(bo, np.float32).reshape(1, 1),
    }
    in_maps = []
    for i in range(ncores):
        sl = slice(i * BS, (i + 1) * BS)
        m = dict(common)
        m["xt"] = _pack_x8(x[sl], t_steps, SX)
        m["tdt"] = _pack_x8(time_delta[sl], t_steps, STD)
        in_maps.append(m)
    return in_maps


def run(inputs, trace=False, trace_kwargs=None):
    from concourse.bass_utils import run_bass_kernel_spmd

    nc = build_program()
    in_maps = prepare_in_maps(**inputs)
    res = run_bass_kernel_spmd(
        nc, in_maps, list(range(NCORES)), trace=trace,
        trace_kwargs=trace_kwargs or {},
    )
    outs = np.concatenate(
        [np.asarray(res.results[i]["out"]) for i in range(NCORES)], axis=0
    ).astype(np.float32)
    return outs, res


def kernel(**inputs):
    outs, _ = run(inputs, trace=False)
    return outs


# revision 5
# speedup vs baseline: 1.7507x; 1.7507x over previous
"""Trainium2 Bass kernel for nn_Discriminator (GRU-like recurrent discriminator).

Math (per batch row):
    belta = exp(-relu(td @ Wb^T + bb))                       # (T, H)
    for t in 0..T-1:
        s = belta[t] * s
        u = sigmoid(s @ W1h^T + x[t] @ W1x^T + b1)
        r = sigmoid(s @ W2h^T + x[t] @ W2x^T + b2)
        n = tanh((r*s) @ W3h^T + x[t] @ W3x^T + b3)
        s = (1-u)*s + u*n
    out = sigmoid(s @ Wo^T + bo)

Strategy: data-parallel over 8 cores on the batch dim (B=256 -> 32/core).
Phase 1 (belta + per-gate x-contributions) is computed blockwise (16
steps/block) in fp8 DoubleRow matmuls (contraction 256/instruction,
0.5 cycles/row; the slow fp8 weight load amortizes over 512-col
streams) into SBUF ring buffers; the Tile list-scheduler interleaves
this work into the recurrence's dependency stalls.  Phase 2 (the
sequential T-scan) uses bf16 weight-stationary matmuls (bf16 has the
4-rows/cycle fast weight load; fp8 loads 8x slower per byte and loses
at 32-col streams).  The f32 state-carry path is exact; matmul
operands are bf16.  Per-step PSUM injection of the x-contributions is
5 wide identity matmuls instead of 24 narrow ones.
"""

import numpy as np
import ml_dtypes

B, T, IN, H = 256, 96, 512, 1024
NCORES = 8
BS = B // NCORES      # 32 batch rows per core
HC = H // 128         # 8 hidden chunks
KC = IN // 128        # 4 input chunks
CB = HC * BS          # 256 packed columns: col = chunk*BS + b
H2 = CB // 2          # 128 packed cols per half
KH = HC // 2          # 4 chunks per half

TS = 16               # time steps per phase-1 block
SC = TS * BS          # 512 psum cols per phase-1 tile
NSLOT = 2             # ring slots

# fp8 scaling for phase-1 only (ml_dtypes.float8_e4m3: max 240)
SW = 2.0 ** 12        # x/belta weight scale
SX = 2.0 ** 5         # x scale (randn, clipped)
STD = 2.0 ** 7        # time_delta scale ([0,1))
PS1_URN = SW * SX     # 2^17: phase-1 psum scale for u/r/n jobs
PS1_B = SW * STD      # 2^19: phase-1 psum scale for belta job

BF16 = ml_dtypes.bfloat16
FP8 = ml_dtypes.float8_e4m3


def build_program(t_steps=T):
    import concourse.mybir as mybir
    import concourse.tile as tile
    from concourse import bacc
    from concourse.masks import make_identity

    f32 = mybir.dt.float32
    bf16 = mybir.dt.bfloat16
    f8 = mybir.dt.float8e4
    AF = mybir.ActivationFunctionType
    ALU = mybir.AluOpType
    DR = mybir.MatmulPerfMode.DoubleRow
    TB = t_steps * BS
    NS = t_steps // TS    # number of phase-1 blocks

    nc = bacc.Bacc("TRN2", target_bir_lowering=False)

    # ---- DRAM I/O (per core; weights replicated by the host) ----
    xt = nc.dram_tensor("xt", [KC, 128, TB], f8, kind="ExternalInput")
    tdt = nc.dram_tensor("tdt", [KC, 128, TB], f8, kind="ExternalInput")
    # recurrent weights, bf16: [p, k, m*128+j] = W[m*128+j, k*128+p]
    w1h = nc.dram_tensor("w1h", [128, HC, H], bf16, kind="ExternalInput")
    w2h = nc.dram_tensor("w2h", [128, HC, H], bf16, kind="ExternalInput")
    w3h = nc.dram_tensor("w3h", [128, HC, H], bf16, kind="ExternalInput")
    # x weights, fp8 DoubleRow layout (KC/2 = 2 double-chunks)
    w1x = nc.dram_tensor("w1x", [128, 2, KC // 2, H], f8, kind="ExternalInput")
    w2x = nc.dram_tensor("w2x", [128, 2, KC // 2, H], f8, kind="ExternalInput")
    w3x = nc.dram_tensor("w3x", [128, 2, KC // 2, H], f8, kind="ExternalInput")
    wbt = nc.dram_tensor("wbt", [128, 2, KC // 2, H], f8, kind="ExternalInput")
    # biases: b1..b3 true scale; bbs = bb * PS1_B (for the pre-exp relu)
    b1t = nc.dram_tensor("b1t", [128, HC], f32, kind="ExternalInput")
    b2t = nc.dram_tensor("b2t", [128, HC], f32, kind="ExternalInput")
    b3t = nc.dram_tensor("b3t", [128, HC], f32, kind="ExternalInput")
    bbs = nc.dram_tensor("bbs", [128, HC], f32, kind="ExternalInput")
    wot = nc.dram_tensor("wot", [128, HC], f32, kind="ExternalInput")
    bot = nc.dram_tensor("bot", [1, 1], f32, kind="ExternalInput")
    out = nc.dram_tensor("out", [BS, 1], f32, kind="ExternalOutput")

    with tile.TileContext(nc) as tc:
        with (
            tc.tile_pool(name="singles", bufs=1) as S,
            tc.tile_pool(name="scp", bufs=2) as scp,
            tc.tile_pool(name="ps2", bufs=1, space="PSUM") as ps2,
            tc.tile_pool(name="ps1p", bufs=3, space="PSUM") as ps1p,
        ):
            # ---- persistent SBUF ----
            sw1h = S.tile([128, HC, H], bf16)
            sw2h = S.tile([128, HC, H], bf16)
            sw3h = S.tile([128, HC, H], bf16)
            sw1x = S.tile([128, 2, KC // 2, H], f8)
            sw2x = S.tile([128, 2, KC // 2, H], f8)
            sw3x = S.tile([128, 2, KC // 2, H], f8)
            swbt = S.tile([128, 2, KC // 2, H], f8)
            sb1 = S.tile([128, HC], f32)
            sb2 = S.tile([128, HC], f32)
            sb3 = S.tile([128, HC], f32)
            sbbs = S.tile([128, HC], f32)
            swo = S.tile([128, HC], f32)
            sbo = S.tile([1, 1], f32)
            ident = S.tile([128, 128], bf16)
            make_identity(nc, ident)

            # rings: gate x-contributions (+bias, true scale) + belta
            ring_u = S.tile([128, NSLOT, TS, CB], bf16)
            ring_r = S.tile([128, NSLOT, TS, CB], bf16)
            ring_n = S.tile([128, NSLOT, TS, CB], bf16)
            ring_b = S.tile([128, NSLOT, TS, CB], f32)
            # x/td stream rings (fp8, scaled)
            xr = S.tile([128, NSLOT, KC, SC], f8)
            tdr = S.tile([128, NSLOT, KC, SC], f8)

            # state: f32 carry + bf16 decayed state
            st_lo = S.tile([128, H2], f32)
            st_hi = S.tile([128, H2], f32)
            nc.vector.memset(st_lo, 0.0)
            nc.vector.memset(st_hi, 0.0)

            # ---- upfront DMAs ----
            nc.sync.dma_start(out=sbbs, in_=bbs[:, :])
            nc.sync.dma_start(out=swbt, in_=wbt[:, :, :, :])
            nc.sync.dma_start(out=sb1, in_=b1t[:, :])
            nc.sync.dma_start(out=sb2, in_=b2t[:, :])
            nc.sync.dma_start(out=sb3, in_=b3t[:, :])
            nc.sync.dma_start(out=sw1x, in_=w1x[:, :, :, :])
            nc.sync.dma_start(out=sw2x, in_=w2x[:, :, :, :])
            nc.sync.dma_start(out=sw3x, in_=w3x[:, :, :, :])
            nc.sync.dma_start(out=sw1h, in_=w1h[:, :, :])
            nc.sync.dma_start(out=sw2h, in_=w2h[:, :, :])
            nc.sync.dma_start(out=sw3h, in_=w3h[:, :, :])
            nc.sync.dma_start(out=swo, in_=wot[:, :])
            nc.sync.dma_start(out=sbo, in_=bot[:, :])

            # ---- phase-1 machinery (fp8 DoubleRow) ----
            def block_dmas(s):
                sl = s % NSLOT
                for k in range(KC):
                    nc.sync.dma_start(
                        out=tdr[:, sl, k, :], in_=tdt[k, :, s * SC:(s + 1) * SC]
                    )
                    nc.sync.dma_start(
                        out=xr[:, sl, k, :], in_=xt[k, :, s * SC:(s + 1) * SC]
                    )

            def emit_unit(s, jobi, m):
                """One m-chunk of one job of block s: 2 DR matmuls + post."""
                sl = s % NSLOT
                ps = ps1p.tile([128, SC], f32, tag="ps1", name="ps1")
                wsb, rin = (
                    (swbt, tdr), (sw1x, xr), (sw2x, xr), (sw3x, xr)
                )[jobi]
                for c2 in range(KC // 2):
                    nc.tensor.matmul(
                        ps,
                        wsb[:, :, c2, m * 128:(m + 1) * 128],
                        rin[:, sl, 2 * c2:2 * c2 + 2, :],
                        start=(c2 == 0), stop=(c2 == KC // 2 - 1),
                        perf_mode=DR,
                    )
                ps3 = ps.rearrange("p (t b) -> p t b", b=BS)
                if jobi == 0:
                    # belta = exp(-relu(z + bb)); psum = z * PS1_B
                    tmp = scp.tile([128, SC], f32, tag="p1b", name="p1b")
                    nc.scalar.activation(
                        tmp, ps, AF.Relu, bias=sbbs[:, m:m + 1], scale=1.0
                    )
                    t3 = tmp.rearrange("p (t b) -> p t b", b=BS)
                    nc.scalar.activation(
                        ring_b[:, sl, :, m * BS:(m + 1) * BS], t3, AF.Exp,
                        scale=-1.0 / PS1_B,
                    )
                else:
                    bias = (None, sb1, sb2, sb3)[jobi]
                    oview = (None, ring_u, ring_r, ring_n)[jobi][
                        :, sl, :, m * BS:(m + 1) * BS
                    ]
                    nc.vector.tensor_scalar(
                        oview, ps3, 1.0 / PS1_URN, bias[:, m:m + 1],
                        op0=ALU.mult, op1=ALU.add,
                    )

            def feed_block_units(s, lo, hi):
                """Emit units [lo, hi) of block s (unit = jobi*HC + m),
                belta job first so next-block decay factors are ready."""
                for ui in range(lo, hi):
                    emit_unit(s, ui // HC, ui % HC)

            # ---- prologue: block 0 ----
            block_dmas(0)
            feed_block_units(0, 0, 4 * HC)

            # ---- recurrence ----
            sbb = S.tile([128, HC, BS], bf16)    # belta * state (matmul rhs)
            nc.vector.memset(sbb, 0.0)
            sbbf = sbb.rearrange("p c b -> p (c b)")

            for t in range(t_steps):
                sl = (t // TS) % NSLOT
                tt = t % TS
                ru = ring_u[:, sl, tt, :]
                rr = ring_r[:, sl, tt, :]
                rn = ring_n[:, sl, tt, :]

                psr = ps2.tile([128, CB], f32, tag="psr", name="psr")
                psu_lo = ps2.tile([128, H2], f32, tag="psu_lo", name="psu_lo")
                psu_hi = ps2.tile([128, H2], f32, tag="psu_hi", name="psu_hi")
                psn_lo = ps2.tile([128, H2], f32, tag="psn_lo", name="psn_lo")
                psn_hi = ps2.tile([128, H2], f32, tag="psn_hi", name="psn_hi")

                # inject x-contributions (+bias)
                nc.tensor.matmul(psr, ident, rr, start=True, stop=False)
                nc.tensor.matmul(psu_lo, ident, ru[:, :H2], start=True, stop=False)
                nc.tensor.matmul(psu_hi, ident, ru[:, H2:], start=True, stop=False)
                nc.tensor.matmul(psn_lo, ident, rn[:, :H2], start=True, stop=False)
                nc.tensor.matmul(psn_hi, ident, rn[:, H2:], start=True, stop=False)

                # r gate: k-outer so the low state half unblocks it
                for k in range(HC):
                    for m in range(HC):
                        nc.tensor.matmul(
                            psr[:, m * BS:(m + 1) * BS],
                            sw2h[:, k, m * 128:(m + 1) * 128],
                            sbb[:, k, :],
                            start=False,
                            stop=(k == HC - 1 and m == HC - 1),
                        )
                rg = scp.tile([128, CB], bf16, tag="rg", name="rg")
                nc.scalar.activation(rg, psr, AF.Sigmoid)
                rs = scp.tile([128, HC, BS], bf16, tag="rs", name="rs")
                rsf = rs.rearrange("p c b -> p (c b)")
                nc.vector.tensor_mul(rsf, rg, sbbf)

                # u gate: low-half psum group completes first
                for mg in range(2):
                    for k in range(HC):
                        for m in range(mg * KH, mg * KH + KH):
                            nc.tensor.matmul(
                                (psu_lo, psu_hi)[mg][
                                    :, (m - mg * KH) * BS:(m - mg * KH + 1) * BS
                                ],
                                sw1h[:, k, m * 128:(m + 1) * 128],
                                sbb[:, k, :],
                                start=False,
                                stop=(k == HC - 1 and m == mg * KH + KH - 1),
                            )
                # n gate (rhs = r*s)
                for mg in range(2):
                    for k in range(HC):
                        for m in range(mg * KH, mg * KH + KH):
                            nc.tensor.matmul(
                                (psn_lo, psn_hi)[mg][
                                    :, (m - mg * KH) * BS:(m - mg * KH + 1) * BS
                                ],
                                sw3h[:, k, m * 128:(m + 1) * 128],
                                rs[:, k, :],
                                start=False,
                                stop=(k == HC - 1 and m == mg * KH + KH - 1),
                            )

                last = t == t_steps - 1
                if not last:
                    t1 = t + 1
                    rb = ring_b[:, (t1 // TS) % NSLOT, t1 % TS, :]

                for half in range(2):
                    lo, hi = half * H2, (half + 1) * H2
                    psu = (psu_lo, psu_hi)[half]
                    psn = (psn_lo, psn_hi)[half]
                    st = (st_lo, st_hi)[half]
                    ug = scp.tile([128, H2], bf16, tag=f"ug{half}", name="ug")
                    nc.scalar.activation(ug, psu, AF.Sigmoid)
                    # stm = belta*state (f32) on gpsimd (off critical engines)
                    if not last:
                        stm = scp.tile([128, H2], f32, tag=f"stm{half}", name="stm")
                        nc.gpsimd.tensor_mul(stm, st, rb[:, lo:hi])
                    ng = scp.tile([128, H2], bf16, tag=f"ng{half}", name="ng")
                    nc.scalar.activation(ng, psn, AF.Tanh)
                    e = scp.tile([128, H2], bf16, tag=f"e{half}", name="e")
                    nc.vector.tensor_mul(e, ug, ng)
                    wneg = scp.tile([128, H2], f32, tag=f"wn{half}", name="wneg")
                    if last:
                        # st = e - (ug-1)*st  [no decay after the last step]
                        nc.vector.scalar_tensor_tensor(
                            wneg, ug, 1.0, st, op0=ALU.subtract, op1=ALU.mult
                        )
                        nc.vector.tensor_sub(st, e, wneg)
                    else:
                        # wneg = (ug-1)*stm ; st' = e - wneg ; sbb' = st'*rb
                        nc.vector.scalar_tensor_tensor(
                            wneg, ug, 1.0, stm, op0=ALU.subtract, op1=ALU.mult
                        )
                        nc.vector.tensor_sub(st, e, wneg)
                        nc.vector.tensor_mul(sbbf[:, lo:hi], st, rb[:, lo:hi])

                # ---- feed next phase-1 block into this step's stalls ----
                fb = t // TS + 1
                if fb < NS:
                    ph = t % TS
                    if ph == 0:
                        block_dmas(fb)
                    feed_block_units(fb, min(3 * ph, 4 * HC),
                                     min(3 * (ph + 1), 4 * HC))

            # ---- head: out = sigmoid(s @ Wo^T + bo) ----
            pso = ps2.tile([1, BS], f32, tag="psr", name="pso")
            stl3 = st_lo.rearrange("p (c b) -> p c b", b=BS)
            sth3 = st_hi.rearrange("p (c b) -> p c b", b=BS)
            for k in range(HC):
                src = stl3[:, k, :] if k < KH else sth3[:, k - KH, :]
                nc.tensor.matmul(
                    pso, swo[:, k:k + 1], src,
                    start=(k == 0), stop=(k == HC - 1),
                )
            ob = scp.tile([1, BS], f32, tag="ob", name="ob")
            nc.scalar.activation(ob, pso, AF.Sigmoid, bias=sbo[0:1, 0:1])
            nc.sync.dma_start(out=out[:, :], in_=ob)

    nc.finalize()
    return nc


def _f8(a, clip=224.0):
    return np.clip(a, -clip, clip).astype(FP8)


def _pack_wh(w):  # [H, H] -> [128, HC, H];  out[p,k,m*128+j] = w[m*128+j,k*128+p]
    return np.ascontiguousarray(
        w.reshape(HC, 128, HC, 128).transpose(3, 2, 0, 1).reshape(128, HC, H)
    ).astype(BF16)


def _pack_wx8(w):
    # [H, IN] -> [128, 2, KC//2, H]; out[p,d,c2,m*128+j] = w[m*128+j,(2c2+d)*128+p]*SW
    t = (w * SW).reshape(HC, 128, KC // 2, 2, 128).transpose(4, 3, 2, 0, 1)
    return _f8(np.ascontiguousarray(t.reshape(128, 2, KC // 2, H)))


def _pack_bias(b, scale=1.0):  # [H] -> [128, HC]
    return np.ascontiguousarray((b * scale).reshape(HC, 128).T).astype(np.float32)


def _pack_x8(xs, t_steps, scale):  # [BS, t, IN] -> [KC, 128, t*BS]
    t = (xs * scale).reshape(BS, t_steps, KC, 128).transpose(2, 3, 1, 0)
    return _f8(np.ascontiguousarray(t.reshape(KC, 128, -1)))


def prepare_in_maps(x, time_delta, Wb, bb, W1, b1, W2, b2, W3, b3, Wo, bo,
                    t_steps=T, ncores=NCORES):
    x = np.asarray(x, np.float32)
    time_delta = np.asarray(time_delta, np.float32)
    common = {
        "w1h": _pack_wh(np.asarray(W1, np.float32)[:, :H]),
        "w2h": _pack_wh(np.asarray(W2, np.float32)[:, :H]),
        "w3h": _pack_wh(np.asarray(W3, np.float32)[:, :H]),
        "w1x": _pack_wx8(np.asarray(W1, np.float32)[:, H:]),
        "w2x": _pack_wx8(np.asarray(W2, np.float32)[:, H:]),
        "w3x": _pack_wx8(np.asarray(W3, np.float32)[:, H:]),
        "wbt": _pack_wx8(np.asarray(Wb, np.float32)),
        "b1t": _pack_bias(np.asarray(b1, np.float32)),
        "b2t": _pack_bias(np.asarray(b2, np.float32)),
        "b3t": _pack_bias(np.asarray(b3, np.float32)),
        "bbs": _pack_bias(np.asarray(bb, np.float32), PS1_B),
        "wot": _pack_bias(np.asarray(Wo, np.float32).reshape(H)),
        "bot": np.asarray(bo, np.float32).reshape(1, 1),
    }
    in_maps = []
    for i in range(ncores):
        sl = slice(i * BS, (i + 1) * BS)
        m = dict(common)
        m["xt"] = _pack_x8(x[sl], t_steps, SX)
        m["tdt"] = _pack_x8(time_delta[sl], t_steps, STD)
        in_maps.append(m)
    return in_maps


def run(inputs, trace=False, trace_kwargs=None):
    from concourse.bass_utils import run_bass_kernel_spmd

    nc = build_program()
    in_maps = prepare_in_maps(**inputs)
    res = run_bass_kernel_spmd(
        nc, in_maps, list(range(NCORES)), trace=trace,
        trace_kwargs=trace_kwargs or {},
    )
    outs = np.concatenate(
        [np.asarray(res.results[i]["out"]) for i in range(NCORES)], axis=0
    ).astype(np.float32)
    return outs, res


def kernel(**inputs):
    outs, _ = run(inputs, trace=False)
    return outs


# revision 6
# speedup vs baseline: 1.8002x; 1.0283x over previous
"""Trainium2 Bass kernel for nn_Discriminator (GRU-like recurrent discriminator).

Math (per batch row):
    belta = exp(-relu(td @ Wb^T + bb))                       # (T, H)
    for t in 0..T-1:
        s = belta[t] * s
        u = sigmoid(s @ W1h^T + x[t] @ W1x^T + b1)
        r = sigmoid(s @ W2h^T + x[t] @ W2x^T + b2)
        n = tanh((r*s) @ W3h^T + x[t] @ W3x^T + b3)
        s = (1-u)*s + u*n
    out = sigmoid(s @ Wo^T + bo)

Strategy: data-parallel over 8 cores on the batch dim (B=256 -> 32/core).
Phase 1 (belta + per-gate x-contributions) is computed blockwise (16
steps/block) in fp8 DoubleRow matmuls (contraction 256/instruction,
0.5 cycles/row; the slow fp8 weight load amortizes over 512-col
streams) into SBUF ring buffers; the Tile list-scheduler interleaves
this work into the recurrence's dependency stalls.  Phase 2 (the
sequential T-scan) uses bf16 weight-stationary matmuls (bf16 has the
4-rows/cycle fast weight load; fp8 loads 8x slower per byte and loses
at 32-col streams).  The f32 state-carry path is exact; matmul
operands are bf16.  Per-step PSUM injection of the x-contributions is
5 wide identity matmuls instead of 24 narrow ones.
"""

import numpy as np
import ml_dtypes

B, T, IN, H = 256, 96, 512, 1024
NCORES = 8
BS = B // NCORES      # 32 batch rows per core
HC = H // 128         # 8 hidden chunks
KC = IN // 128        # 4 input chunks
CB = HC * BS          # 256 packed columns: col = chunk*BS + b
H2 = CB // 2          # 128 packed cols per half
KH = HC // 2          # 4 chunks per half

TS = 16               # time steps per phase-1 block
SC = TS * BS          # 512 psum cols per phase-1 tile
NSLOT = 2             # ring slots

# fp8 scaling for phase-1 only (ml_dtypes.float8_e4m3: max 240)
SW = 2.0 ** 12        # x/belta weight scale
SX = 2.0 ** 5         # x scale (randn, clipped)
STD = 2.0 ** 7        # time_delta scale ([0,1))
PS1_URN = SW * SX     # 2^17: phase-1 psum scale for u/r/n jobs
PS1_B = SW * STD      # 2^19: phase-1 psum scale for belta job

BF16 = ml_dtypes.bfloat16
FP8 = ml_dtypes.float8_e4m3


def build_program(t_steps=T):
    import concourse.mybir as mybir
    import concourse.tile as tile
    from concourse import bacc
    from concourse.masks import make_identity

    f32 = mybir.dt.float32
    bf16 = mybir.dt.bfloat16
    f8 = mybir.dt.float8e4
    AF = mybir.ActivationFunctionType
    ALU = mybir.AluOpType
    DR = mybir.MatmulPerfMode.DoubleRow
    TB = t_steps * BS
    NS = t_steps // TS    # number of phase-1 blocks

    nc = bacc.Bacc("TRN2", target_bir_lowering=False)

    # ---- DRAM I/O (per core; weights replicated by the host) ----
    xt = nc.dram_tensor("xt", [KC, 128, TB], f8, kind="ExternalInput")
    tdt = nc.dram_tensor("tdt", [KC, 128, TB], bf16, kind="ExternalInput")
    # recurrent weights, bf16: [p, k, m*128+j] = W[m*128+j, k*128+p]
    w1h = nc.dram_tensor("w1h", [128, HC, H], bf16, kind="ExternalInput")
    w2h = nc.dram_tensor("w2h", [128, HC, H], bf16, kind="ExternalInput")
    w3h = nc.dram_tensor("w3h", [128, HC, H], bf16, kind="ExternalInput")
    # x weights, fp8 DoubleRow layout (KC/2 = 2 double-chunks)
    w1x = nc.dram_tensor("w1x", [128, 2, KC // 2, H], f8, kind="ExternalInput")
    w2x = nc.dram_tensor("w2x", [128, 2, KC // 2, H], f8, kind="ExternalInput")
    w3x = nc.dram_tensor("w3x", [128, 2, KC // 2, H], f8, kind="ExternalInput")
    wbt = nc.dram_tensor("wbt", [128, KC, H], bf16, kind="ExternalInput")
    # biases: b1..b3 true scale; bbs = bb * PS1_B (for the pre-exp relu)
    b1t = nc.dram_tensor("b1t", [128, HC], f32, kind="ExternalInput")
    b2t = nc.dram_tensor("b2t", [128, HC], f32, kind="ExternalInput")
    b3t = nc.dram_tensor("b3t", [128, HC], f32, kind="ExternalInput")
    bbs = nc.dram_tensor("bbs", [128, HC], f32, kind="ExternalInput")
    wot = nc.dram_tensor("wot", [128, HC], f32, kind="ExternalInput")
    bot = nc.dram_tensor("bot", [1, 1], f32, kind="ExternalInput")
    out = nc.dram_tensor("out", [BS, 1], f32, kind="ExternalOutput")

    with tile.TileContext(nc) as tc:
        with (
            tc.tile_pool(name="singles", bufs=1) as S,
            tc.tile_pool(name="scp", bufs=2) as scp,
            tc.tile_pool(name="ps2", bufs=1, space="PSUM") as ps2,
            tc.tile_pool(name="ps1p", bufs=2, space="PSUM") as ps1p,
        ):
            # ---- persistent SBUF ----
            sw1h = S.tile([128, HC, H], bf16)
            sw2h = S.tile([128, HC, H], bf16)
            sw3h = S.tile([128, HC, H], bf16)
            sw1x = S.tile([128, 2, KC // 2, H], f8)
            sw2x = S.tile([128, 2, KC // 2, H], f8)
            sw3x = S.tile([128, 2, KC // 2, H], f8)
            swbt = S.tile([128, KC, H], bf16)
            sb1 = S.tile([128, HC], f32)
            sb2 = S.tile([128, HC], f32)
            sb3 = S.tile([128, HC], f32)
            sbbs = S.tile([128, HC], f32)
            swo = S.tile([128, HC], f32)
            sbo = S.tile([1, 1], f32)
            ident = S.tile([128, 128], bf16)
            make_identity(nc, ident)

            # rings: gate x-contributions (+bias, true scale) + belta
            ring_u = S.tile([128, NSLOT, TS, CB], bf16)
            ring_r = S.tile([128, NSLOT, TS, CB], bf16)
            ring_n = S.tile([128, NSLOT, TS, CB], bf16)
            ring_b = S.tile([128, NSLOT, TS, CB], f32)
            # x/td stream rings (fp8, scaled)
            xr = S.tile([128, NSLOT, KC, SC], f8)
            tdr = S.tile([128, NSLOT, KC, SC], bf16)

            # state: f32 carry + bf16 decayed state
            st_lo = S.tile([128, H2], f32)
            st_hi = S.tile([128, H2], f32)
            nc.vector.memset(st_lo, 0.0)
            nc.vector.memset(st_hi, 0.0)

            # ---- upfront DMAs ----
            nc.sync.dma_start(out=sbbs, in_=bbs[:, :])
            nc.sync.dma_start(out=swbt, in_=wbt[:, :, :])
            nc.sync.dma_start(out=sb1, in_=b1t[:, :])
            nc.sync.dma_start(out=sb2, in_=b2t[:, :])
            nc.sync.dma_start(out=sb3, in_=b3t[:, :])
            nc.sync.dma_start(out=sw1x, in_=w1x[:, :, :, :])
            nc.sync.dma_start(out=sw2x, in_=w2x[:, :, :, :])
            nc.sync.dma_start(out=sw3x, in_=w3x[:, :, :, :])
            nc.sync.dma_start(out=sw1h, in_=w1h[:, :, :])
            nc.sync.dma_start(out=sw2h, in_=w2h[:, :, :])
            nc.sync.dma_start(out=sw3h, in_=w3h[:, :, :])
            nc.sync.dma_start(out=swo, in_=wot[:, :])
            nc.sync.dma_start(out=sbo, in_=bot[:, :])

            # ---- phase-1 machinery (fp8 DoubleRow) ----
            def block_dmas(s):
                sl = s % NSLOT
                for k in range(KC):
                    nc.sync.dma_start(
                        out=tdr[:, sl, k, :], in_=tdt[k, :, s * SC:(s + 1) * SC]
                    )
                    nc.sync.dma_start(
                        out=xr[:, sl, k, :], in_=xt[k, :, s * SC:(s + 1) * SC]
                    )

            def emit_unit(s, jobi, m):
                """One m-chunk of one job of block s: 2 DR matmuls + post."""
                sl = s % NSLOT
                ps = ps1p.tile([128, SC], f32, tag="ps1", name="ps1")
                if jobi == 0:
                    # belta job in bf16 (accuracy); relu on DVE keeps the
                    # ACT function table at {Exp, Sigmoid, Tanh}
                    for k in range(KC):
                        nc.tensor.matmul(
                            ps,
                            swbt[:, k, m * 128:(m + 1) * 128],
                            tdr[:, sl, k, :],
                            start=(k == 0), stop=(k == KC - 1),
                        )
                    tmp = scp.tile([128, SC], f32, tag="p1b", name="p1b")
                    nc.vector.tensor_scalar(
                        tmp, ps, sbbs[:, m:m + 1], 0.0,
                        op0=ALU.add, op1=ALU.max,
                    )
                    t3 = tmp.rearrange("p (t b) -> p t b", b=BS)
                    nc.scalar.activation(
                        ring_b[:, sl, :, m * BS:(m + 1) * BS], t3, AF.Exp,
                        scale=-1.0,
                    )
                    return
                wsb, rin = (None, (sw1x, xr), (sw2x, xr), (sw3x, xr))[jobi]
                for c2 in range(KC // 2):
                    nc.tensor.matmul(
                        ps,
                        wsb[:, :, c2, m * 128:(m + 1) * 128],
                        rin[:, sl, 2 * c2:2 * c2 + 2, :],
                        start=(c2 == 0), stop=(c2 == KC // 2 - 1),
                        perf_mode=DR,
                    )
                ps3 = ps.rearrange("p (t b) -> p t b", b=BS)
                if True:
                    bias = (None, sb1, sb2, sb3)[jobi]
                    oview = (None, ring_u, ring_r, ring_n)[jobi][
                        :, sl, :, m * BS:(m + 1) * BS
                    ]
                    nc.vector.tensor_scalar(
                        oview, ps3, 1.0 / PS1_URN, bias[:, m:m + 1],
                        op0=ALU.mult, op1=ALU.add,
                    )

            def feed_block_units(s, lo, hi):
                """Emit units [lo, hi) of block s (unit = jobi*HC + m),
                belta job first so next-block decay factors are ready."""
                for ui in range(lo, hi):
                    emit_unit(s, ui // HC, ui % HC)

            # ---- prologue: block 0 ----
            block_dmas(0)
            feed_block_units(0, 0, 4 * HC)

            # ---- recurrence ----
            sbb = S.tile([128, HC, BS], bf16)    # belta * state (matmul rhs)
            nc.vector.memset(sbb, 0.0)
            sbbf = sbb.rearrange("p c b -> p (c b)")

            for t in range(t_steps):
                sl = (t // TS) % NSLOT
                tt = t % TS
                ru = ring_u[:, sl, tt, :]
                rr = ring_r[:, sl, tt, :]
                rn = ring_n[:, sl, tt, :]

                psr = ps2.tile([128, CB], f32, tag="psr", name="psr")
                psu = ps2.tile([128, CB], f32, tag="psu", name="psu")
                psn_lo = ps2.tile([128, H2], f32, tag="psn_lo", name="psn_lo",
                                  bufs=2)
                psn_hi = ps2.tile([128, H2], f32, tag="psn_hi", name="psn_hi",
                                  bufs=2)

                # inject x-contributions (+bias)
                nc.tensor.matmul(psr, ident, rr, start=True, stop=False)
                nc.tensor.matmul(psu, ident, ru, start=True, stop=False)
                nc.tensor.matmul(psn_lo, ident, rn[:, :H2], start=True, stop=False)
                nc.tensor.matmul(psn_hi, ident, rn[:, H2:], start=True, stop=False)

                # r gate: k-outer so the low state half unblocks it
                for k in range(HC):
                    for m in range(HC):
                        nc.tensor.matmul(
                            psr[:, m * BS:(m + 1) * BS],
                            sw2h[:, k, m * 128:(m + 1) * 128],
                            sbb[:, k, :],
                            start=False,
                            stop=(k == HC - 1 and m == HC - 1),
                        )
                rg = scp.tile([128, CB], bf16, tag="rg", name="rg")
                nc.scalar.activation(rg, psr, AF.Sigmoid)
                rs = scp.tile([128, HC, BS], bf16, tag="rs", name="rs")
                rsf = rs.rearrange("p c b -> p (c b)")
                nc.vector.tensor_mul(rsf, rg, sbbf)

                # u gate (single psum bank, single sigmoid)
                for k in range(HC):
                    for m in range(HC):
                        nc.tensor.matmul(
                            psu[:, m * BS:(m + 1) * BS],
                            sw1h[:, k, m * 128:(m + 1) * 128],
                            sbb[:, k, :],
                            start=False,
                            stop=(k == HC - 1 and m == HC - 1),
                        )
                # n gate (rhs = r*s)
                for mg in range(2):
                    for k in range(HC):
                        for m in range(mg * KH, mg * KH + KH):
                            nc.tensor.matmul(
                                (psn_lo, psn_hi)[mg][
                                    :, (m - mg * KH) * BS:(m - mg * KH + 1) * BS
                                ],
                                sw3h[:, k, m * 128:(m + 1) * 128],
                                rs[:, k, :],
                                start=False,
                                stop=(k == HC - 1 and m == mg * KH + KH - 1),
                            )

                last = t == t_steps - 1
                if not last:
                    t1 = t + 1
                    rb = ring_b[:, (t1 // TS) % NSLOT, t1 % TS, :]

                ug = scp.tile([128, CB], bf16, tag="ug", name="ug")
                nc.scalar.activation(ug, psu, AF.Sigmoid)
                for half in range(2):
                    lo, hi = half * H2, (half + 1) * H2
                    psn = (psn_lo, psn_hi)[half]
                    st = (st_lo, st_hi)[half]
                    ugh = ug[:, lo:hi]
                    # stm = belta*state (f32) on gpsimd (off critical engines)
                    if not last:
                        stm = scp.tile([128, H2], f32, tag=f"stm{half}", name="stm")
                        nc.gpsimd.tensor_mul(stm, st, rb[:, lo:hi])
                    ng = scp.tile([128, H2], bf16, tag=f"ng{half}", name="ng")
                    nc.scalar.activation(ng, psn, AF.Tanh)
                    e = scp.tile([128, H2], bf16, tag=f"e{half}", name="e")
                    nc.vector.tensor_mul(e, ugh, ng)
                    wneg = scp.tile([128, H2], f32, tag=f"wn{half}", name="wneg")
                    if last:
                        # st = e - (ug-1)*st  [no decay after the last step]
                        nc.vector.scalar_tensor_tensor(
                            wneg, ugh, 1.0, st, op0=ALU.subtract, op1=ALU.mult
                        )
                        nc.vector.tensor_sub(st, e, wneg)
                    else:
                        # wneg = (ug-1)*stm ; st' = e - wneg ; sbb' = st'*rb
                        nc.vector.scalar_tensor_tensor(
                            wneg, ugh, 1.0, stm, op0=ALU.subtract, op1=ALU.mult
                        )
                        nc.vector.tensor_sub(st, e, wneg)
                        nc.gpsimd.tensor_mul(sbbf[:, lo:hi], st, rb[:, lo:hi])

                # ---- feed next phase-1 block into this step's stalls ----
                fb = t // TS + 1
                if fb < NS:
                    ph = t % TS
                    if ph == 0:
                        block_dmas(fb)
                    with tc.high_priority(offset=-450):
                        feed_block_units(fb, min(3 * ph, 4 * HC),
                                         min(3 * (ph + 1), 4 * HC))

            # ---- head: out = sigmoid(s @ Wo^T + bo) ----
            pso = ps2.tile([1, BS], f32, tag="psr", name="pso")
            stl3 = st_lo.rearrange("p (c b) -> p c b", b=BS)
            sth3 = st_hi.rearrange("p (c b) -> p c b", b=BS)
            for k in range(HC):
                src = stl3[:, k, :] if k < KH else sth3[:, k - KH, :]
                nc.tensor.matmul(
                    pso, swo[:, k:k + 1], src,
                    start=(k == 0), stop=(k == HC - 1),
                )
            ob = scp.tile([1, BS], f32, tag="ob", name="ob")
            nc.scalar.activation(ob, pso, AF.Sigmoid, bias=sbo[0:1, 0:1])
            nc.sync.dma_start(out=out[:, :], in_=ob)

    nc.finalize()
    return nc


def _f8(a, clip=224.0):
    return np.clip(a, -clip, clip).astype(FP8)


def _pack_wh(w):  # [H, H] -> [128, HC, H];  out[p,k,m*128+j] = w[m*128+j,k*128+p]
    return np.ascontiguousarray(
        w.reshape(HC, 128, HC, 128).transpose(3, 2, 0, 1).reshape(128, HC, H)
    ).astype(BF16)


def _pack_wx8(w):
    # [H, IN] -> [128, 2, KC//2, H]; out[p,d,c2,m*128+j] = w[m*128+j,(2c2+d)*128+p]*SW
    t = (w * SW).reshape(HC, 128, KC // 2, 2, 128).transpose(4, 3, 2, 0, 1)
    return _f8(np.ascontiguousarray(t.reshape(128, 2, KC // 2, H)))


def _pack_wx(w):  # [H, IN] -> [128, KC, H] bf16
    return np.ascontiguousarray(
        w.reshape(HC, 128, KC, 128).transpose(3, 2, 0, 1).reshape(128, KC, H)
    ).astype(BF16)


def _pack_x(xs, t_steps):  # [BS, t, IN] -> [KC, 128, t*BS] bf16
    return np.ascontiguousarray(
        xs.reshape(BS, t_steps, KC, 128).transpose(2, 3, 1, 0).reshape(KC, 128, -1)
    ).astype(BF16)


def _pack_bias(b, scale=1.0):  # [H] -> [128, HC]
    return np.ascontiguousarray((b * scale).reshape(HC, 128).T).astype(np.float32)


def _pack_x8(xs, t_steps, scale):  # [BS, t, IN] -> [KC, 128, t*BS]
    t = (xs * scale).reshape(BS, t_steps, KC, 128).transpose(2, 3, 1, 0)
    return _f8(np.ascontiguousarray(t.reshape(KC, 128, -1)))


def prepare_in_maps(x, time_delta, Wb, bb, W1, b1, W2, b2, W3, b3, Wo, bo,
                    t_steps=T, ncores=NCORES):
    x = np.asarray(x, np.float32)
    time_delta = np.asarray(time_delta, np.float32)
    common = {
        "w1h": _pack_wh(np.asarray(W1, np.float32)[:, :H]),
        "w2h": _pack_wh(np.asarray(W2, np.float32)[:, :H]),
        "w3h": _pack_wh(np.asarray(W3, np.float32)[:, :H]),
        "w1x": _pack_wx8(np.asarray(W1, np.float32)[:, H:]),
        "w2x": _pack_wx8(np.asarray(W2, np.float32)[:, H:]),
        "w3x": _pack_wx8(np.asarray(W3, np.float32)[:, H:]),
        "wbt": _pack_wx(np.asarray(Wb, np.float32)),
        "b1t": _pack_bias(np.asarray(b1, np.float32)),
        "b2t": _pack_bias(np.asarray(b2, np.float32)),
        "b3t": _pack_bias(np.asarray(b3, np.float32)),
        "bbs": _pack_bias(np.asarray(bb, np.float32)),
        "wot": _pack_bias(np.asarray(Wo, np.float32).reshape(H)),
        "bot": np.asarray(bo, np.float32).reshape(1, 1),
    }
    in_maps = []
    for i in range(ncores):
        sl = slice(i * BS, (i + 1) * BS)
        m = dict(common)
        m["xt"] = _pack_x8(x[sl], t_steps, SX)
        m["tdt"] = _pack_x(time_delta[sl], t_steps)
        in_maps.append(m)
    return in_maps


def run(inputs, trace=False, trace_kwargs=None):
    from concourse.bass_utils import run_bass_kernel_spmd

    nc = build_program()
    in_maps = prepare_in_maps(**inputs)
    res = run_bass_kernel_spmd(
        nc, in_maps, list(range(NCORES)), trace=trace,
        trace_kwargs=trace_kwargs or {},
    )
    outs = np.concatenate(
        [np.asarray(res.results[i]["out"]) for i in range(NCORES)], axis=0
    ).astype(np.float32)
    return outs, res


def kernel(**inputs):
    outs, _ = run(inputs, trace=False)
    return outs


# revision 8
# speedup vs baseline: 1.8459x; 1.0253x over previous
"""Trainium2 Bass kernel for nn_Discriminator (GRU-like recurrent discriminator).

Math (per batch row):
    belta = exp(-relu(td @ Wb^T + bb))                       # (T, H)
    for t in 0..T-1:
        s = belta[t] * s
        u = sigmoid(s @ W1h^T + x[t] @ W1x^T + b1)
        r = sigmoid(s @ W2h^T + x[t] @ W2x^T + b2)
        n = tanh((r*s) @ W3h^T + x[t] @ W3x^T + b3)
        s = (1-u)*s + u*n
    out = sigmoid(s @ Wo^T + bo)

Strategy: data-parallel over 8 cores on the batch dim (B=256 -> 32/core).
Phase 1 (belta + per-gate x-contributions) is computed blockwise (16
steps/block) in fp8 DoubleRow matmuls (contraction 256/instruction,
0.5 cycles/row; the slow fp8 weight load amortizes over 512-col
streams) into SBUF ring buffers; the Tile list-scheduler interleaves
this work into the recurrence's dependency stalls.  Phase 2 (the
sequential T-scan) uses bf16 weight-stationary matmuls (bf16 has the
4-rows/cycle fast weight load; fp8 loads 8x slower per byte and loses
at 32-col streams).  The f32 state-carry path is exact; matmul
operands are bf16.  Per-step PSUM injection of the x-contributions is
5 wide identity matmuls instead of 24 narrow ones.
"""

import numpy as np
import ml_dtypes

B, T, IN, H = 256, 96, 512, 1024
NCORES = 8
BS = B // NCORES      # 32 batch rows per core
HC = H // 128         # 8 hidden chunks
KC = IN // 128        # 4 input chunks
CB = HC * BS          # 256 packed columns: col = chunk*BS + b
H2 = CB // 2          # 128 packed cols per half
KH = HC // 2          # 4 chunks per half

TS = 16               # time steps per phase-1 block
SC = TS * BS          # 512 psum cols per phase-1 tile
NSLOT = 2             # ring slots

# fp8 scaling for phase-1 only (ml_dtypes.float8_e4m3: max 240)
SW = 2.0 ** 12        # x/belta weight scale
SX = 2.0 ** 5         # x scale (randn, clipped)
STD = 2.0 ** 7        # time_delta scale ([0,1))
PS1_URN = SW * SX     # 2^17: phase-1 psum scale for u/r/n jobs
PS1_B = SW * STD      # 2^19: phase-1 psum scale for belta job

BF16 = ml_dtypes.bfloat16
FP8 = ml_dtypes.float8_e4m3


def build_program(t_steps=T):
    import concourse.mybir as mybir
    import concourse.tile as tile
    from concourse import bacc
    from concourse.masks import make_identity

    f32 = mybir.dt.float32
    bf16 = mybir.dt.bfloat16
    f8 = mybir.dt.float8e4
    AF = mybir.ActivationFunctionType
    ALU = mybir.AluOpType
    DR = mybir.MatmulPerfMode.DoubleRow
    TB = t_steps * BS
    NS = t_steps // TS    # number of phase-1 blocks

    nc = bacc.Bacc("TRN2", target_bir_lowering=False)

    # ---- DRAM I/O (per core; weights replicated by the host) ----
    xt = nc.dram_tensor("xt", [KC, 128, TB], f8, kind="ExternalInput")
    tdt = nc.dram_tensor("tdt", [KC, 128, TB], bf16, kind="ExternalInput")
    # recurrent weights, bf16: [p, k, m*128+j] = W[m*128+j, k*128+p]
    w1h = nc.dram_tensor("w1h", [128, HC, H], bf16, kind="ExternalInput")
    w2h = nc.dram_tensor("w2h", [128, HC, H], bf16, kind="ExternalInput")
    w3h = nc.dram_tensor("w3h", [128, HC, H], bf16, kind="ExternalInput")
    # x weights, fp8 DoubleRow layout (KC/2 = 2 double-chunks)
    w1x = nc.dram_tensor("w1x", [128, 2, KC // 2, H], f8, kind="ExternalInput")
    w2x = nc.dram_tensor("w2x", [128, 2, KC // 2, H], f8, kind="ExternalInput")
    w3x = nc.dram_tensor("w3x", [128, 2, KC // 2, H], f8, kind="ExternalInput")
    wbt = nc.dram_tensor("wbt", [128, KC, H], bf16, kind="ExternalInput")
    # biases: b1..b3 true scale; bbs = bb * PS1_B (for the pre-exp relu)
    b1t = nc.dram_tensor("b1t", [128, HC], f32, kind="ExternalInput")
    b2t = nc.dram_tensor("b2t", [128, HC], f32, kind="ExternalInput")
    b3t = nc.dram_tensor("b3t", [128, HC], f32, kind="ExternalInput")
    bbs = nc.dram_tensor("bbs", [128, HC], f32, kind="ExternalInput")
    wot = nc.dram_tensor("wot", [128, HC], f32, kind="ExternalInput")
    bot = nc.dram_tensor("bot", [1, 1], f32, kind="ExternalInput")
    out = nc.dram_tensor("out", [BS, 1], f32, kind="ExternalOutput")

    with tile.TileContext(nc) as tc:
        with (
            tc.tile_pool(name="singles", bufs=1) as S,
            tc.tile_pool(name="scp", bufs=2) as scp,
            tc.tile_pool(name="ps2", bufs=1, space="PSUM") as ps2,
            tc.tile_pool(name="ps1p", bufs=2, space="PSUM") as ps1p,
        ):
            # ---- persistent SBUF ----
            sw1h = S.tile([128, HC, H], bf16)
            sw2h = S.tile([128, HC, H], bf16)
            sw3h = S.tile([128, HC, H], bf16)
            sw1x = S.tile([128, 2, KC // 2, H], f8)
            sw2x = S.tile([128, 2, KC // 2, H], f8)
            sw3x = S.tile([128, 2, KC // 2, H], f8)
            swbt = S.tile([128, KC, H], bf16)
            sb1 = S.tile([128, HC], f32)
            sb2 = S.tile([128, HC], f32)
            sb3 = S.tile([128, HC], f32)
            sbbs = S.tile([128, HC], f32)
            swo = S.tile([128, HC], f32)
            sbo = S.tile([1, 1], f32)
            ident = S.tile([128, 128], bf16)
            make_identity(nc, ident)

            # rings: gate x-contributions (+bias, true scale) + belta
            ring_u = S.tile([128, NSLOT, TS, CB], bf16)
            ring_r = S.tile([128, NSLOT, TS, CB], bf16)
            ring_n = S.tile([128, NSLOT, TS, CB], bf16)
            ring_b = S.tile([128, NSLOT, TS, CB], f32)
            # x/td stream rings (fp8, scaled)
            xr = S.tile([128, NSLOT, KC, SC], f8)
            tdr = S.tile([128, NSLOT, KC, SC], bf16)

            # state: f32 carry + bf16 decayed state
            st_lo = S.tile([128, H2], f32)
            st_hi = S.tile([128, H2], f32)
            nc.vector.memset(st_lo, 0.0)
            nc.vector.memset(st_hi, 0.0)

            # ---- upfront DMAs (phase-1 block-0 inputs first so the PE
            # starts ~5us in instead of waiting behind 6MB of wh weights) ----
            nc.sync.dma_start(out=sbbs, in_=bbs[:, :])
            nc.sync.dma_start(out=swbt, in_=wbt[:, :, :])

            # ---- phase-1 machinery (fp8 DoubleRow) ----
            def block_dmas(s):
                sl = s % NSLOT
                for k in range(KC):
                    nc.sync.dma_start(
                        out=tdr[:, sl, k, :], in_=tdt[k, :, s * SC:(s + 1) * SC]
                    )
                    nc.sync.dma_start(
                        out=xr[:, sl, k, :], in_=xt[k, :, s * SC:(s + 1) * SC]
                    )

            def emit_unit(s, jobi, m):
                """One m-chunk of one job of block s: 2 DR matmuls + post."""
                sl = s % NSLOT
                ps = ps1p.tile([128, SC], f32, tag="ps1", name="ps1")
                if jobi == 0:
                    # belta job in bf16 (accuracy); relu on DVE keeps the
                    # ACT function table at {Exp, Sigmoid, Tanh}
                    for k in range(KC):
                        nc.tensor.matmul(
                            ps,
                            swbt[:, k, m * 128:(m + 1) * 128],
                            tdr[:, sl, k, :],
                            start=(k == 0), stop=(k == KC - 1),
                        )
                    tmp = scp.tile([128, SC], f32, tag="p1b", name="p1b")
                    nc.vector.tensor_scalar(
                        tmp, ps, sbbs[:, m:m + 1], 0.0,
                        op0=ALU.add, op1=ALU.max,
                    )
                    t3 = tmp.rearrange("p (t b) -> p t b", b=BS)
                    nc.scalar.activation(
                        ring_b[:, sl, :, m * BS:(m + 1) * BS], t3, AF.Exp,
                        scale=-1.0,
                    )
                    return
                wsb, rin = (None, (sw1x, xr), (sw2x, xr), (sw3x, xr))[jobi]
                for c2 in range(KC // 2):
                    nc.tensor.matmul(
                        ps,
                        wsb[:, :, c2, m * 128:(m + 1) * 128],
                        rin[:, sl, 2 * c2:2 * c2 + 2, :],
                        start=(c2 == 0), stop=(c2 == KC // 2 - 1),
                        perf_mode=DR,
                    )
                ps3 = ps.rearrange("p (t b) -> p t b", b=BS)
                if True:
                    bias = (None, sb1, sb2, sb3)[jobi]
                    oview = (None, ring_u, ring_r, ring_n)[jobi][
                        :, sl, :, m * BS:(m + 1) * BS
                    ]
                    nc.vector.tensor_scalar(
                        oview, ps3, 1.0 / PS1_URN, bias[:, m:m + 1],
                        op0=ALU.mult, op1=ALU.add,
                    )

            def feed_block_units(s, lo, hi):
                """Emit units [lo, hi) of block s (unit = jobi*HC + m),
                belta job first so next-block decay factors are ready."""
                for ui in range(lo, hi):
                    emit_unit(s, ui // HC, ui % HC)

            # ---- prologue: block 0 ----
            block_dmas(0)
            nc.sync.dma_start(out=sb1, in_=b1t[:, :])
            nc.sync.dma_start(out=sb2, in_=b2t[:, :])
            nc.sync.dma_start(out=sb3, in_=b3t[:, :])
            nc.sync.dma_start(out=sw1x, in_=w1x[:, :, :, :])
            nc.sync.dma_start(out=sw2x, in_=w2x[:, :, :, :])
            nc.sync.dma_start(out=sw3x, in_=w3x[:, :, :, :])
            nc.sync.dma_start(out=sw1h, in_=w1h[:, :, :])
            nc.sync.dma_start(out=sw2h, in_=w2h[:, :, :])
            nc.sync.dma_start(out=sw3h, in_=w3h[:, :, :])
            nc.sync.dma_start(out=swo, in_=wot[:, :])
            nc.sync.dma_start(out=sbo, in_=bot[:, :])
            feed_block_units(0, 0, 4 * HC)

            # ---- recurrence ----
            sbb = S.tile([128, HC, BS], bf16)    # belta * state (matmul rhs)
            nc.vector.memset(sbb, 0.0)
            sbbf = sbb.rearrange("p c b -> p (c b)")

            for t in range(t_steps):
                sl = (t // TS) % NSLOT
                tt = t % TS
                ru = ring_u[:, sl, tt, :]
                rr = ring_r[:, sl, tt, :]
                rn = ring_n[:, sl, tt, :]

                psr = ps2.tile([128, CB], f32, tag="psr", name="psr")
                psu = ps2.tile([128, CB], f32, tag="psu", name="psu")
                psn_lo = ps2.tile([128, H2], f32, tag="psn_lo", name="psn_lo",
                                  bufs=2)
                psn_hi = ps2.tile([128, H2], f32, tag="psn_hi", name="psn_hi",
                                  bufs=2)

                # inject x-contributions (+bias)
                nc.tensor.matmul(psr, ident, rr, start=True, stop=False)
                nc.tensor.matmul(psu, ident, ru, start=True, stop=False)
                nc.tensor.matmul(psn_lo, ident, rn[:, :H2], start=True, stop=False)
                nc.tensor.matmul(psn_hi, ident, rn[:, H2:], start=True, stop=False)

                # r gate: k-outer so the low state half unblocks it
                for k in range(HC):
                    for m in range(HC):
                        nc.tensor.matmul(
                            psr[:, m * BS:(m + 1) * BS],
                            sw2h[:, k, m * 128:(m + 1) * 128],
                            sbb[:, k, :],
                            start=False,
                            stop=(k == HC - 1 and m == HC - 1),
                        )
                rg = scp.tile([128, CB], bf16, tag="rg", name="rg")
                nc.scalar.activation(rg, psr, AF.Sigmoid)
                rs = scp.tile([128, HC, BS], bf16, tag="rs", name="rs")
                rsf = rs.rearrange("p c b -> p (c b)")
                nc.vector.tensor_mul(rsf, rg, sbbf)

                # u gate (single psum bank, single sigmoid)
                for k in range(HC):
                    for m in range(HC):
                        nc.tensor.matmul(
                            psu[:, m * BS:(m + 1) * BS],
                            sw1h[:, k, m * 128:(m + 1) * 128],
                            sbb[:, k, :],
                            start=False,
                            stop=(k == HC - 1 and m == HC - 1),
                        )
                # n gate (rhs = r*s)
                for mg in range(2):
                    for k in range(HC):
                        for m in range(mg * KH, mg * KH + KH):
                            nc.tensor.matmul(
                                (psn_lo, psn_hi)[mg][
                                    :, (m - mg * KH) * BS:(m - mg * KH + 1) * BS
                                ],
                                sw3h[:, k, m * 128:(m + 1) * 128],
                                rs[:, k, :],
                                start=False,
                                stop=(k == HC - 1 and m == mg * KH + KH - 1),
                            )

                last = t == t_steps - 1
                if not last:
                    t1 = t + 1
                    rb = ring_b[:, (t1 // TS) % NSLOT, t1 % TS, :]

                ug = scp.tile([128, CB], bf16, tag="ug", name="ug")
                nc.scalar.activation(ug, psu, AF.Sigmoid)
                for half in range(2):
                    lo, hi = half * H2, (half + 1) * H2
                    psn = (psn_lo, psn_hi)[half]
                    st = (st_lo, st_hi)[half]
                    ugh = ug[:, lo:hi]
                    # stm = belta*state (f32) on gpsimd (off critical engines)
                    if not last:
                        stm = scp.tile([128, H2], f32, tag=f"stm{half}", name="stm")
                        nc.gpsimd.tensor_mul(stm, st, rb[:, lo:hi])
                    ng = scp.tile([128, H2], bf16, tag=f"ng{half}", name="ng")
                    nc.scalar.activation(ng, psn, AF.Tanh)
                    e = scp.tile([128, H2], bf16, tag=f"e{half}", name="e")
                    nc.vector.tensor_mul(e, ugh, ng)
                    wneg = scp.tile([128, H2], f32, tag=f"wn{half}", name="wneg")
                    if last:
                        # st = e - (ug-1)*st  [no decay after the last step]
                        nc.vector.scalar_tensor_tensor(
                            wneg, ugh, 1.0, st, op0=ALU.subtract, op1=ALU.mult
                        )
                        nc.vector.tensor_sub(st, e, wneg)
                    else:
                        # wneg = (ug-1)*stm ; st' = e - wneg ; sbb' = st'*rb
                        nc.vector.scalar_tensor_tensor(
                            wneg, ugh, 1.0, stm, op0=ALU.subtract, op1=ALU.mult
                        )
                        nc.vector.tensor_sub(st, e, wneg)
                        nc.gpsimd.tensor_mul(sbbf[:, lo:hi], st, rb[:, lo:hi])

                # ---- feed next phase-1 block into this step's stalls ----
                fb = t // TS + 1
                if fb < NS:
                    ph = t % TS
                    if ph == 0:
                        block_dmas(fb)
                    if ph < 8:
                        ulo, uhi = 8 + 3 * ph, 8 + 3 * (ph + 1)
                    elif ph < 12:
                        ulo, uhi = 2 * (ph - 8), 2 * (ph - 8) + 2
                    else:
                        ulo = uhi = 0
                    with tc.high_priority(offset=-450):
                        for ui in range(ulo, uhi):
                            emit_unit(fb, ui // HC, ui % HC)

            # ---- head: out = sigmoid(s @ Wo^T + bo) ----
            pso = ps2.tile([1, BS], f32, tag="psr", name="pso")
            stl3 = st_lo.rearrange("p (c b) -> p c b", b=BS)
            sth3 = st_hi.rearrange("p (c b) -> p c b", b=BS)
            for k in range(HC):
                src = stl3[:, k, :] if k < KH else sth3[:, k - KH, :]
                nc.tensor.matmul(
                    pso, swo[:, k:k + 1], src,
                    start=(k == 0), stop=(k == HC - 1),
                )
            ob = scp.tile([1, BS], f32, tag="ob", name="ob")
            nc.scalar.activation(ob, pso, AF.Sigmoid, bias=sbo[0:1, 0:1])
            nc.sync.dma_start(out=out[:, :], in_=ob)

    nc.finalize()
    return nc


def _f8(a, clip=224.0):
    return np.clip(a, -clip, clip).astype(FP8)


def _pack_wh(w):  # [H, H] -> [128, HC, H];  out[p,k,m*128+j] = w[m*128+j,k*128+p]
    return np.ascontiguousarray(
        w.reshape(HC, 128, HC, 128).transpose(3, 2, 0, 1).reshape(128, HC, H)
    ).astype(BF16)


def _pack_wx8(w):
    # [H, IN] -> [128, 2, KC//2, H]; out[p,d,c2,m*128+j] = w[m*128+j,(2c2+d)*128+p]*SW
    t = (w * SW).reshape(HC, 128, KC // 2, 2, 128).transpose(4, 3, 2, 0, 1)
    return _f8(np.ascontiguousarray(t.reshape(128, 2, KC // 2, H)))


def _pack_wx(w):  # [H, IN] -> [128, KC, H] bf16
    return np.ascontiguousarray(
        w.reshape(HC, 128, KC, 128).transpose(3, 2, 0, 1).reshape(128, KC, H)
    ).astype(BF16)


def _pack_x(xs, t_steps):  # [BS, t, IN] -> [KC, 128, t*BS] bf16
    return np.ascontiguousarray(
        xs.reshape(BS, t_steps, KC, 128).transpose(2, 3, 1, 0).reshape(KC, 128, -1)
    ).astype(BF16)


def _pack_bias(b, scale=1.0):  # [H] -> [128, HC]
    return np.ascontiguousarray((b * scale).reshape(HC, 128).T).astype(np.float32)


def _pack_x8(xs, t_steps, scale):  # [BS, t, IN] -> [KC, 128, t*BS]
    t = (xs * scale).reshape(BS, t_steps, KC, 128).transpose(2, 3, 1, 0)
    return _f8(np.ascontiguousarray(t.reshape(KC, 128, -1)))


def prepare_in_maps(x, time_delta, Wb, bb, W1, b1, W2, b2, W3, b3, Wo, bo,
                    t_steps=T, ncores=NCORES):
    x = np.asarray(x, np.float32)
    time_delta = np.asarray(time_delta, np.float32)
    common = {
        "w1h": _pack_wh(np.asarray(W1, np.float32)[:, :H]),
        "w2h": _pack_wh(np.asarray(W2, np.float32)[:, :H]),
        "w3h": _pack_wh(np.asarray(W3, np.float32)[:, :H]),
        "w1x": _pack_wx8(np.asarray(W1, np.float32)[:, H:]),
        "w2x": _pack_wx8(np.asarray(W2, np.float32)[:, H:]),
        "w3x": _pack_wx8(np.asarray(W3, np.float32)[:, H:]),
        "wbt": _pack_wx(np.asarray(Wb, np.float32)),
        "b1t": _pack_bias(np.asarray(b1, np.float32)),
        "b2t": _pack_bias(np.asarray(b2, np.float32)),
        "b3t": _pack_bias(np.asarray(b3, np.float32)),
        "bbs": _pack_bias(np.asarray(bb, np.float32)),
        "wot": _pack_bias(np.asarray(Wo, np.float32).reshape(H)),
        "bot": np.asarray(bo, np.float32).reshape(1, 1),
    }
    in_maps = []
    for i in range(ncores):
        sl = slice(i * BS, (i + 1) * BS)
        m = dict(common)
        m["xt"] = _pack_x8(x[sl], t_steps, SX)
        m["tdt"] = _pack_x(time_delta[sl], t_steps)
        in_maps.append(m)
    return in_maps


def run(inputs, trace=False, trace_kwargs=None):
    from concourse.bass_utils import run_bass_kernel_spmd

    nc = build_program()
    in_maps = prepare_in_maps(**inputs)
    res = run_bass_kernel_spmd(
        nc, in_maps, list(range(NCORES)), trace=trace,
        trace_kwargs=trace_kwargs or {},
    )
    outs = np.concatenate(
        [np.asarray(res.results[i]["out"]) for i in range(NCORES)], axis=0
    ).astype(np.float32)
    return outs, res


def kernel(**inputs):
    outs, _ = run(inputs, trace=False)
    return outs
